# revision 16
# baseline (speedup 1.0000x reference)
"""Trainium2 Bass kernel for nn_BalanceLabelAugmentation2 (topk_masking).

Math (reference, restructured; matmul is linear over the mixup):
  For pair (copy c, unlabeled row i) with labeled partner j = idx_c[i]:
    l    = 0.7*Z_o[j] + b + 0.3*Z_u[i]        (Z = feat @ W.T)
    ce   = logsumexp(l) - (0.7*l[label_j] + 0.3*l[pred_i])
  pred/score from the W_o head on feat_u; w = group[pred] & score>thr
  out = sum(ce*w) / max(sum w, 1)

Design (v3): the HOST pre-gathers partner feature rows per pair (input
prep: row duplication + fp8 cast) so the device runs dense fp8 DoubleRow
matmuls over the 5*2048 pair rows per core -- no logit-table AllGather,
no GpSimd descriptor generation.  Class->pair transposition happens on
the DMA xbar (SBUF->SBUF bf16 dma_start_transpose), not the PE.

  per core r (data-parallel over unlabeled rows, pairs ordered c-major
  n = c*2048 + g*128 + p so every 4-chunk tile shares one copy c and
  4 consecutive u-chunks g):
    u-head:  [0.3*s3*W | s_o*W_o](fp8) @ Xu(fp8) -> [128,512] PSUM
             -> ACT unscale+bias -> bf16 -> xbar transpose
             -> lpu[t][128, 4, 128]  (cols 0:51 zu'=0.3Zu+b, 64:115 lo)
             per chunk: pred-onehot, score/group weights (DVE/ACT)
    pairs:   (0.7*s*W)(fp8) @ G(fp8) -> [64,512] PSUM -> ACT unscale
             -> bf16 -> xbar transpose -> lpz[128, 4, 64]
             lp = lpz + lpu[.,.,0:51]  (DVE bf16 2x)
             ce: nm=-max (DVE), 4x Exp(bias=nm) (ACT), yw/pw (DVE bf16),
             d1/dot reduces (DVE)
  final: per-core [ce_sum, w_sum] -> AllGather -> scalar on every core.

fp8 e4m3 on the feature side (clip +-240, TRN max), bf16 intermediate
logits.  Weight scales ship as an input column so the compiled program
is input-independent.  Measured end-to-end vs f32 reference: ~7e-4 rel.
"""

import numpy as np
import ml_dtypes

import concourse.bass as bass
import concourse.tile as tile
from concourse import bacc, mybir
from concourse.bass_utils import run_bass_kernel_spmd

F32 = mybir.dt.float32
BF16 = mybir.dt.bfloat16
F8 = mybir.dt.float8e4
AF = mybir.ActivationFunctionType
ALU = mybir.AluOpType
AX = mybir.AxisListType
DR = mybir.MatmulPerfMode.DoubleRow
E4NP = ml_dtypes.float8_e4m3   # TRN-style e4m3, max +-240


class Cfg:
    def __init__(self, n_o=16384, n_u=16384, d=1024, cores=8):
        self.n_o, self.n_u, self.d, self.cores = n_o, n_u, d, cores
        self.c = 51
        self.s = n_o // cores           # labeled rows per core
        self.u = n_u // cores           # unlabeled rows per core
        self.kc = d // 128              # contraction chunks (8)
        self.chunks = self.u // 128     # unlabeled 128-row chunks (16)
        self.utile = self.u // 512      # u-head 512-col tiles (4)
        self.pairs = 5 * self.u         # 10240
        self.nslab = 4                  # G slabs
        self.slab = self.pairs // self.nslab       # 2560 pairs per slab
        self.wtc = 64 + self.c          # W_o head at partition 64


def _ap(tile_ap, offset_ap, pattern):
    """AP on tile_ap's tensor at offset_ap's offset with a custom free pattern."""
    return bass.AP(tensor=tile_ap.tensor, offset=offset_ap.offset,
                   ap=[tile_ap.ap[0]] + pattern)


def build_bass(cfg: Cfg):
    C, KC = cfg.c, cfg.kc
    W5 = cfg.chunks * 5                 # 80 (c,g) chunks
    nc = bacc.Bacc("TRN2", target_bir_lowering=False, debug=False,
                   num_devices=cfg.cores)

    # free layout [nslab, KC, slab] flattened
    g_h = nc.dram_tensor("g", [128, cfg.nslab * KC * cfg.slab], F8,
                         kind="ExternalInput")
    xu_h = nc.dram_tensor("xu", [128, KC * cfg.u], F8, kind="ExternalInput")
    wp_h = nc.dram_tensor("wp", [128, KC * 64], F8, kind="ExternalInput")
    wt_h = nc.dram_tensor("wt", [128, KC * 128], F8, kind="ExternalInput")
    sb2_h = nc.dram_tensor("sb2", [128, 3], F32, kind="ExternalInput")
    consts_h = nc.dram_tensor("consts", [128, 2 * C], F32, kind="ExternalInput")
    ohj_h = nc.dram_tensor("ohj", [128, W5 * C], BF16, kind="ExternalInput")
    out_h = nc.dram_tensor("out", [1, 2], F32, kind="ExternalOutput")

    with tile.TileContext(nc) as tc:
        ppcm = tc.tile_pool(name="persist", bufs=1)
        pp_ = ppcm.__enter__()

        def P(shape, dtype, name):
            return pp_.tile(shape, dtype, name=name, tag=name)

        # ---- persistent/constant SBUF (small stuff on scalar queue) ----
        wp_sb = P([128, KC, 64], F8, "wp_sb")
        nc.gpsimd.dma_start(out=wp_sb[:], in_=wp_h[:])
        wt_sb = P([128, KC, 128], F8, "wt_sb")
        nc.gpsimd.dma_start(out=wt_sb[:], in_=wt_h[:])
        sb2_sb = P([128, 3], F32, "sb2_sb")
        nc.gpsimd.dma_start(out=sb2_sb[:], in_=sb2_h[:])
        consts_sb = P([128, 2 * C], F32, "consts_sb")
        nc.gpsimd.dma_start(out=consts_sb[:], in_=consts_h[:])
        gm_r = consts_sb[:, 0:C]
        gt_r = consts_sb[:, C:2 * C]
        ones128 = P([128, 1], F32, "ones128")
        nc.vector.memset(ones128[:], 1.0)

        # all loads ride the SWDGE (gpsimd) queue: its DMA-completion sem
        # lanes are separate from the 8 HWDGE lanes, so the xbar transposes
        # never block on a lane held by a multi-MB load
        xu_sb = P([128, KC, cfg.u], F8, "xu_sb")
        nc.gpsimd.dma_start(out=xu_sb[:], in_=xu_h[:])

        ohj_sb = P([128, W5, C], BF16, "ohj_sb")

        # transposed u-head logits, one per u-tile; cols 0:51 = zu', 64:115 = lo
        lpu = [P([128, 4, 128], BF16, f"lpu{t}") for t in range(cfg.utile)]

        oh0_all = P([128, cfg.chunks, C], BF16, "oh0_all")
        wbuf = P([128, 2, cfg.chunks], F32, "wbuf")
        d1buf = P([128, W5], F32, "d1buf")
        dotbuf = P([128, W5], F32, "dotbuf")
        nmbuf = P([128, W5], F32, "nmbuf")   # -max(l) per pair

        if True:
            with (
                tc.tile_pool(name="gp", bufs=4) as g_pool,
                tc.tile_pool(name="mmu", bufs=2, space="PSUM") as mmu_pool,
                tc.tile_pool(name="mmp", bufs=3, space="PSUM") as mmp_pool,
                tc.tile_pool(name="ztsp", bufs=2) as zts_pool,
                tc.tile_pool(name="zgp", bufs=4) as zg_pool,
                tc.tile_pool(name="lpzp", bufs=4) as lpz_pool,
                tc.tile_pool(name="lp4p", bufs=4) as lp4_pool,
                tc.tile_pool(name="lpsp", bufs=3) as lps_pool,
                tc.tile_pool(name="ewp", bufs=3) as ew_pool,
                tc.tile_pool(name="ywp", bufs=3) as yw_pool,
                tc.tile_pool(name="pwp", bufs=3) as pw_pool,
                tc.tile_pool(name="stat", bufs=12) as stat_pool,
                tc.tile_pool(name="small", bufs=6) as small_pool,
            ):
                # ---- G slabs + ohj on the scalar ring: G0, ohj, G1..G3 ----
                g_tiles = []
                for s in range(cfg.nslab):
                    gt_t = g_pool.tile([128, KC, cfg.slab], F8, tag="g",
                                       name="gt_t")
                    nc.gpsimd.dma_start(
                        out=gt_t[:],
                        in_=g_h[:, s * KC * cfg.slab:(s + 1) * KC * cfg.slab])
                    g_tiles.append(gt_t)
                    if s == 0:
                        nc.gpsimd.dma_start(out=ohj_sb[:], in_=ohj_h[:])

                # ================= Phase B: unlabeled head =================
                for t in range(cfg.utile):
                    zt = mmu_pool.tile([128, 512], F32, tag="mmu", name="zt")
                    for kp in range(KC // 2):
                        nc.tensor.matmul(
                            zt[:], lhsT=wt_sb[:, 2 * kp:2 * kp + 2, :],
                            rhs=xu_sb[:, 2 * kp:2 * kp + 2,
                                      t * 512:(t + 1) * 512],
                            perf_mode=DR,
                            start=(kp == 0), stop=(kp == KC // 2 - 1))
                    zts = zts_pool.tile([128, 512], BF16, tag="zts",
                                        name="zts")
                    # unscale fp8 weight scaling + bias, per-partition; the
                    # zero rows of sb2 blank the padding partitions
                    nc.scalar.activation(zts[:], zt[:], AF.Identity,
                                         bias=sb2_sb[:, 1:2],
                                         scale=sb2_sb[:, 0:1])
                    nc.sync.dma_start_transpose(lpu[t][:], zts[:])
                    for q in range(4):
                        g = 4 * t + q
                        lo = lpu[t][:, q, 64:64 + C]
                        negm = stat_pool.tile([128, 1], F32, tag="st",
                                              name="negm")
                        nc.vector.tensor_reduce(negm[:], lo, axis=AX.X,
                                                op=ALU.max, negate=True)
                        ej = ew_pool.tile([128, C], F32, tag="ew", name="ej")
                        svec = stat_pool.tile([128, 1], F32, tag="st",
                                              name="svec")
                        nc.scalar.activation(ej[:], lo, AF.Exp,
                                             bias=negm[:], scale=1.0,
                                             accum_out=svec[:])
                        nc.vector.tensor_scalar(
                            out=oh0_all[:, g, :], in0=lo, scalar1=negm[:],
                            scalar2=0.0, op0=ALU.add, op1=ALU.is_equal)
                        gvm = stat_pool.tile([128, 1], F32, tag="st",
                                             name="gvm")
                        jm = small_pool.tile([128, C], F32, tag="sm", name="jm")
                        nc.vector.scalar_tensor_tensor(
                            out=jm[:], in0=oh0_all[:, g, :], scalar=1.0,
                            in1=gm_r, op0=ALU.mult, op1=ALU.mult,
                            accum_out=gvm[:])
                        gvt = stat_pool.tile([128, 1], F32, tag="st",
                                             name="gvt")
                        jt = small_pool.tile([128, C], F32, tag="sm", name="jt")
                        nc.vector.scalar_tensor_tensor(
                            out=jt[:], in0=oh0_all[:, g, :], scalar=1.0,
                            in1=gt_r, op0=ALU.mult, op1=ALU.mult,
                            accum_out=gvt[:])
                        nc.vector.scalar_tensor_tensor(
                            out=wbuf[:, 0, g:g + 1], in0=svec[:], scalar=2.0,
                            in1=gvm[:], op0=ALU.is_lt, op1=ALU.mult)
                        nc.vector.scalar_tensor_tensor(
                            out=wbuf[:, 1, g:g + 1], in0=svec[:],
                            scalar=float(1.0 / 0.3), in1=gvt[:],
                            op0=ALU.is_lt, op1=ALU.mult)

                # ================= Pairs =================
                # chunk m = c*16 + g; tile of 4 chunks shares c, spans
                # u-chunks g0..g0+3 = one lpu tile
                for s in range(cfg.nslab):
                    gt_t = g_tiles[s]
                    for ti in range(cfg.slab // 512):
                        tglob = s * (cfg.slab // 512) + ti
                        m0 = 4 * tglob
                        ut = (m0 % cfg.chunks) // 4
                        zp = mmp_pool.tile([64, 512], F32, tag="mmp",
                                           name="zp")
                        for kp in range(KC // 2):
                            nc.tensor.matmul(
                                zp[:], lhsT=wp_sb[:, 2 * kp:2 * kp + 2, :],
                                rhs=gt_t[:, 2 * kp:2 * kp + 2,
                                         ti * 512:(ti + 1) * 512],
                                perf_mode=DR,
                                start=(kp == 0), stop=(kp == KC // 2 - 1))
                        zg = zg_pool.tile([64, 512], BF16, tag="zg", name="zg")
                        nc.scalar.activation(zg[:], zp[:], AF.Identity,
                                             scale=sb2_sb[0:64, 2:3])
                        lpz = lpz_pool.tile([128, 4, 64], BF16, tag="lpz",
                                            name="lpz")
                        nc.sync.dma_start_transpose(lpz[:], zg[:])
                        # lp = Zg^T + zu'  (bf16; Pool engine, DVE relief)
                        lp4 = lp4_pool.tile([128, 4, C], BF16, tag="lp4",
                                            name="lp4")
                        nc.gpsimd.tensor_tensor(
                            out=lp4[:], in0=lpz[:, :, 0:C],
                            in1=lpu[ut][:, :, 0:C], op=ALU.add)
                        nc.vector.tensor_reduce(
                            nmbuf[:, m0:m0 + 4], lp4[:], axis=AX.X,
                            op=ALU.max, negate=True)
                        lps4 = lps_pool.tile([128, 4, C], BF16, tag="lps",
                                             name="lps4")
                        nc.vector.tensor_tensor(
                            out=lps4[:], in0=lp4[:],
                            in1=_ap(nmbuf[:], nmbuf[:, m0:m0 + 4],
                                    [[1, 4], [0, C]]),
                            op=ALU.add)
                        ew4 = ew_pool.tile([128, 4, C], BF16, tag="ew",
                                           name="ew4")
                        nc.scalar.activation(ew4[:], lps4[:], AF.Exp)
                        nc.vector.tensor_reduce(
                            d1buf[:, m0:m0 + 4], ew4[:], axis=AX.X,
                            op=ALU.add)
                        g0 = m0 % cfg.chunks
                        yw4 = yw_pool.tile([128, 4, C], BF16, tag="yw",
                                           name="yw4")
                        nc.vector.scalar_tensor_tensor(
                            out=yw4[:], in0=oh0_all[:, g0:g0 + 4, :],
                            scalar=0.3, in1=ohj_sb[:, m0:m0 + 4, :],
                            op0=ALU.mult, op1=ALU.add)
                        pw4 = pw_pool.tile([128, 4, C], BF16, tag="pw",
                                           name="pw4")
                        nc.gpsimd.tensor_tensor(out=pw4[:], in0=lp4[:],
                                                in1=yw4[:], op=ALU.mult)
                        nc.vector.tensor_reduce(
                            dotbuf[:, m0:m0 + 4], pw4[:], axis=AX.X,
                            op=ALU.add)

                # ================= Final reduction =================
                lse = P([128, W5], F32, "lse")
                nc.scalar.activation(lse[:], d1buf[:], AF.Ln)
                ce = P([128, W5], F32, "ce")
                nc.vector.tensor_tensor(out=ce[:], in0=lse[:], in1=nmbuf[:],
                                        op=ALU.subtract)   # lse + max
                nc.vector.tensor_tensor(out=ce[:], in0=ce[:], in1=dotbuf[:],
                                        op=ALU.subtract)
                # weighted sums; chunk m = c*16+g: mid c=0,1 tail c=2,3,4
                accw = P([128, 2], F32, "accw")
                amid = P([128, 1], F32, "amid")
                jA = P([128, 2, cfg.chunks], F32, "jA")
                wA = _ap(wbuf[:], wbuf[:, 0, :], [[0, 2], [1, cfg.chunks]])
                nc.vector.scalar_tensor_tensor(
                    out=jA[:], in0=ce[:, 0:2 * cfg.chunks], scalar=1.0,
                    in1=wA, op0=ALU.mult, op1=ALU.mult, accum_out=amid[:])
                atail = P([128, 1], F32, "atail")
                jB = P([128, 3, cfg.chunks], F32, "jB")
                wB = _ap(wbuf[:], wbuf[:, 1, :], [[0, 3], [1, cfg.chunks]])
                nc.vector.scalar_tensor_tensor(
                    out=jB[:], in0=ce[:, 2 * cfg.chunks:5 * cfg.chunks],
                    scalar=1.0, in1=wB, op0=ALU.mult, op1=ALU.mult,
                    accum_out=atail[:])
                nc.vector.tensor_tensor(out=accw[:, 0:1], in0=amid[:],
                                        in1=atail[:], op=ALU.add)
                # w_sum = 2*sum(midw) + 3*sum(tailw)
                smid = P([128, 1], F32, "smid")
                nc.vector.tensor_reduce(smid[:], wbuf[:, 0, :], axis=AX.X,
                                        op=ALU.add)
                stail = P([128, 1], F32, "stail")
                nc.vector.tensor_reduce(stail[:], wbuf[:, 1, :], axis=AX.X,
                                        op=ALU.add)
                st3 = P([128, 1], F32, "st3")
                nc.vector.tensor_scalar_mul(st3[:], stail[:], 3.0)
                nc.vector.scalar_tensor_tensor(
                    out=accw[:, 1:2], in0=smid[:], scalar=2.0, in1=st3[:],
                    op0=ALU.mult, op1=ALU.add)
                pp = mmu_pool.tile([1, 2], F32, tag="mmu", name="pp")
                nc.tensor.matmul(pp[:], lhsT=ones128[:], rhs=accw[:],
                                 start=True, stop=True)
                ppsb = P([1, 2], F32, "ppsb")
                nc.vector.tensor_copy(ppsb[:], pp[:])
                # per-core [ce_sum, w_sum]; the host does the 16-float
                # all-reduce (a 64B AllGather costs ~30us of trigger+op
                # latency on this part -- pure tail)
                nc.sync.dma_start(out=out_h[:], in_=ppsb[:])

        ppcm.__exit__(None, None, None)

    nc.compile()
    return nc


def _kshard(mat_T, kc=8):
    """[K, M] -> [128, kc*M] with K split into kc 128-chunks."""
    K, M = mat_T.shape
    return np.ascontiguousarray(
        mat_T.reshape(kc, 128, M).transpose(1, 0, 2).reshape(128, kc * M))


def make_in_maps(cfg: Cfg, feat, label, W_o, b_o, W, b, gm, gt, idx_m, idx_t):
    n_o, C, KC = cfg.n_o, cfg.c, cfg.kc
    feat = np.asarray(feat, np.float32)
    label = np.asarray(label).astype(np.int64)
    W_o = np.asarray(W_o, np.float32)
    W = np.asarray(W, np.float32)
    b_o = np.asarray(b_o, np.float32)
    b = np.asarray(b, np.float32)
    gm = np.asarray(gm).astype(np.float32)
    gt = np.asarray(gt).astype(np.float32)
    idxs = np.concatenate([np.asarray(idx_m), np.asarray(idx_t)],
                          0).astype(np.int64)
    label_o = label[:n_o]

    e4 = lambda x: np.clip(x, -240.0, 240.0).astype(E4NP)
    sW = 0.25 / max(float(np.std(0.7 * W)), 1e-12)
    sW3 = 0.25 / max(float(np.std(0.3 * W)), 1e-12)
    sWo = 0.25 / max(float(np.std(W_o)), 1e-12)

    wp_f = np.zeros((cfg.d, 64), np.float32)
    wp_f[:, 0:C] = np.asarray(e4(0.7 * sW * W), np.float32).T
    wp = np.ascontiguousarray(_kshard(wp_f, KC).astype(E4NP))
    wt_f = np.zeros((cfg.d, 128), np.float32)
    wt_f[:, 0:C] = np.asarray(e4(0.3 * sW3 * W), np.float32).T
    wt_f[:, 64:64 + C] = np.asarray(e4(sWo * W_o), np.float32).T
    wt = np.ascontiguousarray(_kshard(wt_f, KC).astype(E4NP))
    sb2 = np.zeros((128, 3), np.float32)
    sb2[0:C, 0] = 1.0 / sW3
    sb2[64:64 + C, 0] = 1.0 / sWo
    sb2[0:C, 1] = b
    sb2[64:64 + C, 1] = b_o
    sb2[0:64, 2] = 1.0 / sW
    consts = np.ascontiguousarray(np.concatenate(
        [np.tile(gm, (128, 1)), np.tile(gt, (128, 1))], axis=1))

    feat8_o = e4(feat[:n_o])
    feat8_u = e4(feat[n_o:])
    cls = np.arange(C, dtype=np.int64)

    in_maps = []
    for r in range(cfg.cores):
        ju = idxs[:, r * cfg.u:(r + 1) * cfg.u]          # [5, 2048]
        j_seq = ju.reshape(-1)                            # c-major pair order
        A = feat8_o[j_seq]                                # [10240, 1024]
        g_arr = np.ascontiguousarray(
            A.reshape(cfg.nslab, cfg.slab, KC, 128).transpose(3, 0, 2, 1)
            .reshape(128, cfg.nslab * KC * cfg.slab))
        B = feat8_u[r * cfg.u:(r + 1) * cfg.u]            # [2048, 1024]
        xu = np.ascontiguousarray(
            B.reshape(cfg.u, KC, 128).transpose(2, 1, 0).reshape(128, -1))
        labj = label_o[j_seq].reshape(5 * cfg.chunks, 128)  # [m, p]
        ohj = (labj.T[:, :, None] == cls).astype(np.float32) * 0.7
        ohj = np.ascontiguousarray(
            ohj.astype(ml_dtypes.bfloat16).reshape(128, -1))
        in_maps.append(dict(g=g_arr, xu=xu, wp=wp, wt=wt, sb2=sb2,
                            consts=consts, ohj=ohj))
    return in_maps


_CACHE = {}


def _get_nc(cfg: Cfg):
    key = (cfg.n_o, cfg.n_u, cfg.d, cfg.cores)
    if key not in _CACHE:
        _CACHE[key] = build_bass(cfg)
    return _CACHE[key]


def _install_ntff_shim():
    """This image's antenv lacks axon_hooks; recreate it so trace=True works."""
    import sys
    import types
    try:
        from antenv.axon_hooks import get_axon_ntff_profile_hook  # noqa: F401
        return
    except ImportError:
        pass
    try:
        import antenv
        from trn_agent_boot.trn_boot import _ntff_profile_via_ctypes
        h = _ntff_profile_via_ctypes("/opt/axon/libaxon_pjrt.so")
        mod = types.ModuleType("antenv.axon_hooks")
        mod.get_axon_ntff_profile_hook = lambda: h
        mod.set_axon_ntff_profile_hook = lambda hook: None
        sys.modules["antenv.axon_hooks"] = mod
        antenv.axon_hooks = mod
    except Exception:
        pass


def kernel(feat, label, W_o, b_o, W, b, group_mid_mask, group_tail_mask,
           idx_m, idx_t, _trace=False):
    if _trace:
        _install_ntff_shim()
    n_u = int(np.asarray(idx_m).shape[1])
    n_o = int(np.asarray(feat).shape[0]) - n_u
    cfg = Cfg(n_o=n_o, n_u=n_u, d=int(np.asarray(feat).shape[1]))
    in_maps = make_in_maps(cfg, feat, label, W_o, b_o, W, b,
                           group_mid_mask, group_tail_mask, idx_m, idx_t)
    nc = _get_nc(cfg)
    res = run_bass_kernel_spmd(nc, in_maps, core_ids=list(range(cfg.cores)),
                               trace=_trace)
    parts = np.stack([np.asarray(res.results[r]["out"], np.float32).reshape(2)
                      for r in range(cfg.cores)])
    tot = parts.sum(axis=0)
    out = np.float32(tot[0] / max(tot[1], 1.0))
    if _trace:
        return out, res
    return out


# revision 17
# speedup vs baseline: 1.1106x; 1.1106x over previous
"""Trainium2 Bass kernel for nn_BalanceLabelAugmentation2 (topk_masking).

Math (reference, restructured; matmul is linear over the mixup):
  For pair (copy c, unlabeled row i) with labeled partner j = idx_c[i]:
    l    = 0.7*Z_o[j] + b + 0.3*Z_u[i]        (Z = feat @ W.T)
    ce   = logsumexp(l) - (0.7*l[label_j] + 0.3*l[pred_i])
  pred/score from the W_o head on feat_u; w = group[pred] & score>thr
  out = sum(ce*w) / max(sum w, 1)

Design (v3): the HOST pre-gathers partner feature rows per pair (input
prep: row duplication + fp8 cast) so the device runs dense fp8 DoubleRow
matmuls over the 5*2048 pair rows per core -- no logit-table AllGather,
no GpSimd descriptor generation.  Class->pair transposition happens on
the DMA xbar (SBUF->SBUF bf16 dma_start_transpose), not the PE.

  per core r (data-parallel over unlabeled rows, pairs ordered c-major
  n = c*2048 + g*128 + p so every 4-chunk tile shares one copy c and
  4 consecutive u-chunks g):
    u-head:  [0.3*s3*W | s_o*W_o](fp8) @ Xu(fp8) -> [128,512] PSUM
             -> ACT unscale+bias -> bf16 -> xbar transpose
             -> lpu[t][128, 4, 128]  (cols 0:51 zu'=0.3Zu+b, 64:115 lo)
             per chunk: pred-onehot, score/group weights (DVE/ACT)
    pairs:   (0.7*s*W)(fp8) @ G(fp8) -> [64,512] PSUM -> ACT unscale
             -> bf16 -> xbar transpose -> lpz[128, 4, 64]
             lp = lpz + lpu[.,.,0:51]  (DVE bf16 2x)
             ce: nm=-max (DVE), 4x Exp(bias=nm) (ACT), yw/pw (DVE bf16),
             d1/dot reduces (DVE)
  final: per-core [ce_sum, w_sum] -> AllGather -> scalar on every core.

fp8 e4m3 on the feature side (clip +-240, TRN max), bf16 intermediate
logits.  Weight scales ship as an input column so the compiled program
is input-independent.  Measured end-to-end vs f32 reference: ~7e-4 rel.
"""

import numpy as np
import ml_dtypes

import concourse.bass as bass
import concourse.tile as tile
from concourse import bacc, mybir
from concourse.bass_utils import run_bass_kernel_spmd

F32 = mybir.dt.float32
BF16 = mybir.dt.bfloat16
F8 = mybir.dt.float8e4
AF = mybir.ActivationFunctionType
ALU = mybir.AluOpType
AX = mybir.AxisListType
DR = mybir.MatmulPerfMode.DoubleRow
E4NP = ml_dtypes.float8_e4m3   # TRN-style e4m3, max +-240


class Cfg:
    def __init__(self, n_o=16384, n_u=16384, d=1024, cores=8):
        self.n_o, self.n_u, self.d, self.cores = n_o, n_u, d, cores
        self.c = 51
        self.s = n_o // cores           # labeled rows per core
        self.u = n_u // cores           # unlabeled rows per core
        self.kc = d // 128              # contraction chunks (8)
        self.chunks = self.u // 128     # unlabeled 128-row chunks (16)
        self.utile = self.u // 512      # u-head 512-col tiles (4)
        self.pairs = 5 * self.u         # 10240
        self.nslab = 4                  # G slabs
        self.slab = self.pairs // self.nslab       # 2560 pairs per slab
        self.wtc = 64 + self.c          # W_o head at partition 64


def _ap(tile_ap, offset_ap, pattern):
    """AP on tile_ap's tensor at offset_ap's offset with a custom free pattern."""
    return bass.AP(tensor=tile_ap.tensor, offset=offset_ap.offset,
                   ap=[tile_ap.ap[0]] + pattern)


def build_bass(cfg: Cfg):
    C, KC = cfg.c, cfg.kc
    W5 = cfg.chunks * 5                 # 80 (c,g) chunks
    nc = bacc.Bacc("TRN2", target_bir_lowering=False, debug=False,
                   num_devices=cfg.cores)

    # free layout [nslab, KC, slab] flattened
    g_h = nc.dram_tensor("g", [128, cfg.nslab * KC * cfg.slab], F8,
                         kind="ExternalInput")
    xu_h = nc.dram_tensor("xu", [128, KC * cfg.u], F8, kind="ExternalInput")
    wp_h = nc.dram_tensor("wp", [128, KC * 64], F8, kind="ExternalInput")
    wt_h = nc.dram_tensor("wt", [128, KC * 128], F8, kind="ExternalInput")
    sb2_h = nc.dram_tensor("sb2", [128, 3], F32, kind="ExternalInput")
    consts_h = nc.dram_tensor("consts", [128, 2 * C], F32, kind="ExternalInput")
    ohj_h = nc.dram_tensor("ohj", [128, W5 * C], BF16, kind="ExternalInput")
    out_h = nc.dram_tensor("out", [1, 2], F32, kind="ExternalOutput")

    with tile.TileContext(nc) as tc:
        ppcm = tc.tile_pool(name="persist", bufs=1)
        pp_ = ppcm.__enter__()

        def P(shape, dtype, name):
            return pp_.tile(shape, dtype, name=name, tag=name)

        # ---- persistent/constant SBUF (small stuff on scalar queue) ----
        wp_sb = P([128, KC, 64], F8, "wp_sb")
        nc.scalar.dma_start(out=wp_sb[:], in_=wp_h[:])
        wt_sb = P([128, KC, 128], F8, "wt_sb")
        nc.scalar.dma_start(out=wt_sb[:], in_=wt_h[:])
        sb2_sb = P([128, 3], F32, "sb2_sb")
        nc.scalar.dma_start(out=sb2_sb[:], in_=sb2_h[:])
        consts_sb = P([128, 2 * C], F32, "consts_sb")
        nc.scalar.dma_start(out=consts_sb[:], in_=consts_h[:])
        gm_r = consts_sb[:, 0:C]
        gt_r = consts_sb[:, C:2 * C]
        ones128 = P([128, 1], F32, "ones128")
        nc.vector.memset(ones128[:], 1.0)

        # all loads ride the SWDGE (gpsimd) queue: its DMA-completion sem
        # lanes are separate from the 8 HWDGE lanes, so the xbar transposes
        # never block on a lane held by a multi-MB load
        xu_sb = P([128, KC, cfg.u], F8, "xu_sb")
        nc.scalar.dma_start(out=xu_sb[:], in_=xu_h[:])

        ohj_sb = P([128, W5, C], BF16, "ohj_sb")

        # transposed u-head logits, one per u-tile; cols 0:51 = zu', 64:115 = lo
        lpu = [P([128, 4, 128], BF16, f"lpu{t}") for t in range(cfg.utile)]

        oh0_all = P([128, cfg.chunks, C], BF16, "oh0_all")
        wbuf = P([128, 2, cfg.chunks], F32, "wbuf")
        d1buf = P([128, W5], F32, "d1buf")
        dotbuf = P([128, W5], F32, "dotbuf")
        nmbuf = P([128, W5], F32, "nmbuf")   # -max(l) per pair

        if True:
            with (
                tc.tile_pool(name="gp", bufs=4) as g_pool,
                tc.tile_pool(name="mmu", bufs=2, space="PSUM") as mmu_pool,
                tc.tile_pool(name="mmp", bufs=3, space="PSUM") as mmp_pool,
                tc.tile_pool(name="ztsp", bufs=2) as zts_pool,
                tc.tile_pool(name="zgp", bufs=4) as zg_pool,
                tc.tile_pool(name="lpzp", bufs=4) as lpz_pool,
                tc.tile_pool(name="lp4p", bufs=4) as lp4_pool,
                tc.tile_pool(name="lpsp", bufs=3) as lps_pool,
                tc.tile_pool(name="ewp", bufs=3) as ew_pool,
                tc.tile_pool(name="ywp", bufs=3) as yw_pool,
                tc.tile_pool(name="pwp", bufs=3) as pw_pool,
                tc.tile_pool(name="stat", bufs=12) as stat_pool,
                tc.tile_pool(name="small", bufs=6) as small_pool,
            ):
                # ---- G slabs + ohj on the scalar ring: G0, ohj, G1..G3 ----
                g_tiles = []
                for s in range(cfg.nslab):
                    gt_t = g_pool.tile([128, KC, cfg.slab], F8, tag="g",
                                       name="gt_t")
                    eng = nc.scalar if s < 2 else nc.gpsimd
                    eng.dma_start(
                        out=gt_t[:],
                        in_=g_h[:, s * KC * cfg.slab:(s + 1) * KC * cfg.slab])
                    g_tiles.append(gt_t)
                    if s == 0:
                        nc.gpsimd.dma_start(out=ohj_sb[:], in_=ohj_h[:])

                # ================= Phase B: unlabeled head =================
                for t in range(cfg.utile):
                    zt = mmu_pool.tile([128, 512], F32, tag="mmu", name="zt")
                    for kp in range(KC // 2):
                        nc.tensor.matmul(
                            zt[:], lhsT=wt_sb[:, 2 * kp:2 * kp + 2, :],
                            rhs=xu_sb[:, 2 * kp:2 * kp + 2,
                                      t * 512:(t + 1) * 512],
                            perf_mode=DR,
                            start=(kp == 0), stop=(kp == KC // 2 - 1))
                    zts = zts_pool.tile([128, 512], BF16, tag="zts",
                                        name="zts")
                    # unscale fp8 weight scaling + bias, per-partition; the
                    # zero rows of sb2 blank the padding partitions
                    nc.scalar.activation(zts[:], zt[:], AF.Identity,
                                         bias=sb2_sb[:, 1:2],
                                         scale=sb2_sb[:, 0:1])
                    nc.sync.dma_start_transpose(lpu[t][:], zts[:])
                    for q in range(4):
                        g = 4 * t + q
                        lo = lpu[t][:, q, 64:64 + C]
                        negm = stat_pool.tile([128, 1], F32, tag="st",
                                              name="negm")
                        nc.vector.tensor_reduce(negm[:], lo, axis=AX.X,
                                                op=ALU.max, negate=True)
                        ej = ew_pool.tile([128, C], F32, tag="ew", name="ej")
                        svec = stat_pool.tile([128, 1], F32, tag="st",
                                              name="svec")
                        nc.scalar.activation(ej[:], lo, AF.Exp,
                                             bias=negm[:], scale=1.0,
                                             accum_out=svec[:])
                        nc.vector.tensor_scalar(
                            out=oh0_all[:, g, :], in0=lo, scalar1=negm[:],
                            scalar2=0.0, op0=ALU.add, op1=ALU.is_equal)
                        gvm = stat_pool.tile([128, 1], F32, tag="st",
                                             name="gvm")
                        jm = small_pool.tile([128, C], F32, tag="sm", name="jm")
                        nc.vector.scalar_tensor_tensor(
                            out=jm[:], in0=oh0_all[:, g, :], scalar=1.0,
                            in1=gm_r, op0=ALU.mult, op1=ALU.mult,
                            accum_out=gvm[:])
                        gvt = stat_pool.tile([128, 1], F32, tag="st",
                                             name="gvt")
                        jt = small_pool.tile([128, C], F32, tag="sm", name="jt")
                        nc.vector.scalar_tensor_tensor(
                            out=jt[:], in0=oh0_all[:, g, :], scalar=1.0,
                            in1=gt_r, op0=ALU.mult, op1=ALU.mult,
                            accum_out=gvt[:])
                        nc.vector.scalar_tensor_tensor(
                            out=wbuf[:, 0, g:g + 1], in0=svec[:], scalar=2.0,
                            in1=gvm[:], op0=ALU.is_lt, op1=ALU.mult)
                        nc.vector.scalar_tensor_tensor(
                            out=wbuf[:, 1, g:g + 1], in0=svec[:],
                            scalar=float(1.0 / 0.3), in1=gvt[:],
                            op0=ALU.is_lt, op1=ALU.mult)

                # ================= Pairs =================
                # chunk m = c*16 + g; tile of 4 chunks shares c, spans
                # u-chunks g0..g0+3 = one lpu tile
                for s in range(cfg.nslab):
                    gt_t = g_tiles[s]
                    for ti in range(cfg.slab // 512):
                        tglob = s * (cfg.slab // 512) + ti
                        m0 = 4 * tglob
                        ut = (m0 % cfg.chunks) // 4
                        zp = mmp_pool.tile([64, 512], F32, tag="mmp",
                                           name="zp")
                        for kp in range(KC // 2):
                            nc.tensor.matmul(
                                zp[:], lhsT=wp_sb[:, 2 * kp:2 * kp + 2, :],
                                rhs=gt_t[:, 2 * kp:2 * kp + 2,
                                         ti * 512:(ti + 1) * 512],
                                perf_mode=DR,
                                start=(kp == 0), stop=(kp == KC // 2 - 1))
                        zg = zg_pool.tile([64, 512], BF16, tag="zg", name="zg")
                        nc.scalar.activation(zg[:], zp[:], AF.Identity,
                                             scale=sb2_sb[0:64, 2:3])
                        lpz = lpz_pool.tile([128, 4, 64], BF16, tag="lpz",
                                            name="lpz")
                        nc.sync.dma_start_transpose(lpz[:], zg[:])
                        # lp = Zg^T + zu'  (bf16; Pool engine, DVE relief)
                        lp4 = lp4_pool.tile([128, 4, C], BF16, tag="lp4",
                                            name="lp4")
                        nc.gpsimd.tensor_tensor(
                            out=lp4[:], in0=lpz[:, :, 0:C],
                            in1=lpu[ut][:, :, 0:C], op=ALU.add)
                        nc.vector.tensor_reduce(
                            nmbuf[:, m0:m0 + 4], lp4[:], axis=AX.X,
                            op=ALU.max, negate=True)
                        lps4 = lps_pool.tile([128, 4, C], BF16, tag="lps",
                                             name="lps4")
                        nc.vector.tensor_tensor(
                            out=lps4[:], in0=lp4[:],
                            in1=_ap(nmbuf[:], nmbuf[:, m0:m0 + 4],
                                    [[1, 4], [0, C]]),
                            op=ALU.add)
                        ew4 = ew_pool.tile([128, 4, C], BF16, tag="ew",
                                           name="ew4")
                        nc.scalar.activation(ew4[:], lps4[:], AF.Exp)
                        nc.vector.tensor_reduce(
                            d1buf[:, m0:m0 + 4], ew4[:], axis=AX.X,
                            op=ALU.add)
                        g0 = m0 % cfg.chunks
                        yw4 = yw_pool.tile([128, 4, C], BF16, tag="yw",
                                           name="yw4")
                        nc.vector.scalar_tensor_tensor(
                            out=yw4[:], in0=oh0_all[:, g0:g0 + 4, :],
                            scalar=0.3, in1=ohj_sb[:, m0:m0 + 4, :],
                            op0=ALU.mult, op1=ALU.add)
                        pw4 = pw_pool.tile([128, 4, C], BF16, tag="pw",
                                           name="pw4")
                        nc.gpsimd.tensor_tensor(out=pw4[:], in0=lp4[:],
                                                in1=yw4[:], op=ALU.mult)
                        nc.vector.tensor_reduce(
                            dotbuf[:, m0:m0 + 4], pw4[:], axis=AX.X,
                            op=ALU.add)

                # ================= Final reduction =================
                lse = P([128, W5], F32, "lse")
                nc.scalar.activation(lse[:], d1buf[:], AF.Ln)
                ce = P([128, W5], F32, "ce")
                nc.vector.tensor_tensor(out=ce[:], in0=lse[:], in1=nmbuf[:],
                                        op=ALU.subtract)   # lse + max
                nc.vector.tensor_tensor(out=ce[:], in0=ce[:], in1=dotbuf[:],
                                        op=ALU.subtract)
                # weighted sums; chunk m = c*16+g: mid c=0,1 tail c=2,3,4
                accw = P([128, 2], F32, "accw")
                amid = P([128, 1], F32, "amid")
                jA = P([128, 2, cfg.chunks], F32, "jA")
                wA = _ap(wbuf[:], wbuf[:, 0, :], [[0, 2], [1, cfg.chunks]])
                nc.vector.scalar_tensor_tensor(
                    out=jA[:], in0=ce[:, 0:2 * cfg.chunks], scalar=1.0,
                    in1=wA, op0=ALU.mult, op1=ALU.mult, accum_out=amid[:])
                atail = P([128, 1], F32, "atail")
                jB = P([128, 3, cfg.chunks], F32, "jB")
                wB = _ap(wbuf[:], wbuf[:, 1, :], [[0, 3], [1, cfg.chunks]])
                nc.vector.scalar_tensor_tensor(
                    out=jB[:], in0=ce[:, 2 * cfg.chunks:5 * cfg.chunks],
                    scalar=1.0, in1=wB, op0=ALU.mult, op1=ALU.mult,
                    accum_out=atail[:])
                nc.vector.tensor_tensor(out=accw[:, 0:1], in0=amid[:],
                                        in1=atail[:], op=ALU.add)
                # w_sum = 2*sum(midw) + 3*sum(tailw)
                smid = P([128, 1], F32, "smid")
                nc.vector.tensor_reduce(smid[:], wbuf[:, 0, :], axis=AX.X,
                                        op=ALU.add)
                stail = P([128, 1], F32, "stail")
                nc.vector.tensor_reduce(stail[:], wbuf[:, 1, :], axis=AX.X,
                                        op=ALU.add)
                st3 = P([128, 1], F32, "st3")
                nc.vector.tensor_scalar_mul(st3[:], stail[:], 3.0)
                nc.vector.scalar_tensor_tensor(
                    out=accw[:, 1:2], in0=smid[:], scalar=2.0, in1=st3[:],
                    op0=ALU.mult, op1=ALU.add)
                pp = mmu_pool.tile([1, 2], F32, tag="mmu", name="pp")
                nc.tensor.matmul(pp[:], lhsT=ones128[:], rhs=accw[:],
                                 start=True, stop=True)
                ppsb = P([1, 2], F32, "ppsb")
                nc.vector.tensor_copy(ppsb[:], pp[:])
                # per-core [ce_sum, w_sum]; the host does the 16-float
                # all-reduce (a 64B AllGather costs ~30us of trigger+op
                # latency on this part -- pure tail)
                nc.sync.dma_start(out=out_h[:], in_=ppsb[:])

        ppcm.__exit__(None, None, None)

    nc.compile()
    return nc


def _kshard(mat_T, kc=8):
    """[K, M] -> [128, kc*M] with K split into kc 128-chunks."""
    K, M = mat_T.shape
    return np.ascontiguousarray(
        mat_T.reshape(kc, 128, M).transpose(1, 0, 2).reshape(128, kc * M))


def make_in_maps(cfg: Cfg, feat, label, W_o, b_o, W, b, gm, gt, idx_m, idx_t):
    n_o, C, KC = cfg.n_o, cfg.c, cfg.kc
    feat = np.asarray(feat, np.float32)
    label = np.asarray(label).astype(np.int64)
    W_o = np.asarray(W_o, np.float32)
    W = np.asarray(W, np.float32)
    b_o = np.asarray(b_o, np.float32)
    b = np.asarray(b, np.float32)
    gm = np.asarray(gm).astype(np.float32)
    gt = np.asarray(gt).astype(np.float32)
    idxs = np.concatenate([np.asarray(idx_m), np.asarray(idx_t)],
                          0).astype(np.int64)
    label_o = label[:n_o]

    e4 = lambda x: np.clip(x, -240.0, 240.0).astype(E4NP)
    sW = 0.25 / max(float(np.std(0.7 * W)), 1e-12)
    sW3 = 0.25 / max(float(np.std(0.3 * W)), 1e-12)
    sWo = 0.25 / max(float(np.std(W_o)), 1e-12)

    wp_f = np.zeros((cfg.d, 64), np.float32)
    wp_f[:, 0:C] = np.asarray(e4(0.7 * sW * W), np.float32).T
    wp = np.ascontiguousarray(_kshard(wp_f, KC).astype(E4NP))
    wt_f = np.zeros((cfg.d, 128), np.float32)
    wt_f[:, 0:C] = np.asarray(e4(0.3 * sW3 * W), np.float32).T
    wt_f[:, 64:64 + C] = np.asarray(e4(sWo * W_o), np.float32).T
    wt = np.ascontiguousarray(_kshard(wt_f, KC).astype(E4NP))
    sb2 = np.zeros((128, 3), np.float32)
    sb2[0:C, 0] = 1.0 / sW3
    sb2[64:64 + C, 0] = 1.0 / sWo
    sb2[0:C, 1] = b
    sb2[64:64 + C, 1] = b_o
    sb2[0:64, 2] = 1.0 / sW
    consts = np.ascontiguousarray(np.concatenate(
        [np.tile(gm, (128, 1)), np.tile(gt, (128, 1))], axis=1))

    feat8_o = e4(feat[:n_o])
    feat8_u = e4(feat[n_o:])
    cls = np.arange(C, dtype=np.int64)

    in_maps = []
    for r in range(cfg.cores):
        ju = idxs[:, r * cfg.u:(r + 1) * cfg.u]          # [5, 2048]
        j_seq = ju.reshape(-1)                            # c-major pair order
        A = feat8_o[j_seq]                                # [10240, 1024]
        g_arr = np.ascontiguousarray(
            A.reshape(cfg.nslab, cfg.slab, KC, 128).transpose(3, 0, 2, 1)
            .reshape(128, cfg.nslab * KC * cfg.slab))
        B = feat8_u[r * cfg.u:(r + 1) * cfg.u]            # [2048, 1024]
        xu = np.ascontiguousarray(
            B.reshape(cfg.u, KC, 128).transpose(2, 1, 0).reshape(128, -1))
        labj = label_o[j_seq].reshape(5 * cfg.chunks, 128)  # [m, p]
        ohj = (labj.T[:, :, None] == cls).astype(np.float32) * 0.7
        ohj = np.ascontiguousarray(
            ohj.astype(ml_dtypes.bfloat16).reshape(128, -1))
        in_maps.append(dict(g=g_arr, xu=xu, wp=wp, wt=wt, sb2=sb2,
                            consts=consts, ohj=ohj))
    return in_maps


_CACHE = {}


def _get_nc(cfg: Cfg):
    key = (cfg.n_o, cfg.n_u, cfg.d, cfg.cores)
    if key not in _CACHE:
        _CACHE[key] = build_bass(cfg)
    return _CACHE[key]


def _install_ntff_shim():
    """This image's antenv lacks axon_hooks; recreate it so trace=True works."""
    import sys
    import types
    try:
        from antenv.axon_hooks import get_axon_ntff_profile_hook  # noqa: F401
        return
    except ImportError:
        pass
    try:
        import antenv
        from trn_agent_boot.trn_boot import _ntff_profile_via_ctypes
        h = _ntff_profile_via_ctypes("/opt/axon/libaxon_pjrt.so")
        mod = types.ModuleType("antenv.axon_hooks")
        mod.get_axon_ntff_profile_hook = lambda: h
        mod.set_axon_ntff_profile_hook = lambda hook: None
        sys.modules["antenv.axon_hooks"] = mod
        antenv.axon_hooks = mod
    except Exception:
        pass


def kernel(feat, label, W_o, b_o, W, b, group_mid_mask, group_tail_mask,
           idx_m, idx_t, _trace=False):
    if _trace:
        _install_ntff_shim()
    n_u = int(np.asarray(idx_m).shape[1])
    n_o = int(np.asarray(feat).shape[0]) - n_u
    cfg = Cfg(n_o=n_o, n_u=n_u, d=int(np.asarray(feat).shape[1]))
    in_maps = make_in_maps(cfg, feat, label, W_o, b_o, W, b,
                           group_mid_mask, group_tail_mask, idx_m, idx_t)
    nc = _get_nc(cfg)
    res = run_bass_kernel_spmd(nc, in_maps, core_ids=list(range(cfg.cores)),
                               trace=_trace)
    parts = np.stack([np.asarray(res.results[r]["out"], np.float32).reshape(2)
                      for r in range(cfg.cores)])
    tot = parts.sum(axis=0)
    out = np.float32(tot[0] / max(tot[1], 1.0))
    if _trace:
        return out, res
    return out
